# revision 1
# baseline (speedup 1.0000x reference)
"""CrossAttention Trainium2 kernel.

Problem (hardcoded): B=8, T=256, S=4096, E=512, KV=768, H=8, D=64.
Sharding: data-parallel over B — one batch per NeuronCore (8 cores).

Per-core dataflow (one batch, all layouts staged host-side):
  inputs (bf16 unless noted):
    ctxT  [768, 4096]   = context[b].T          (KV on partitions)
    xT    [512, 256]    = x[b].T
    m01   [128, 32] f32 = 1.0 where key kept, 0.0 where masked (s=sc*128+p)
    wqT   [512, 512]    = Wq.T * D^-0.5  (scale folded, exact pow2)
    wkvT  [768, 1024]   = Wkv.T
    woT   [512, 512]    = Wo.T
    bo_r  [128, 4] f32  = bo.reshape(4,128).T
  device:
    QT    = wqT.T @ xT            -> [512c, 256t]   (c-major, 4 chunks)
    KT    = wkvT[:, :512].T @ ctxT -> [512c, 4096s] (c-major, 4 chunks = head pairs)
    V'    = ctxT.T @ wkvT[:, 512:] -> [4096s, 8h*65] (64 vals + ones col per head),
            rows multiplied by m01 (mask folded into V' => no -inf anywhere)
    scoresT[s,t] per head = KT_h slices as lhsT, QT_h as rhs (K=64, head pair
            packed into PE row groups 0:64 / 64:128)
    expsT = Exp(scoresT)  (no max subtraction needed: |scores| <~ 8)
    PV    = V'_h-as-lhsT @ expsT -> [65, 256] psum; row 64 = softmax denom
    norm  = reciprocal(denom) broadcast via K=1 fp32 matmul; OT = PV * recip
    outT  = woT.T @ OT + bo -> [512e, 256t] -> host transposes back.

ctx DMA is quartered along S and kv-proj consumption follows arrival order.
Scores for 4 s-chunks of one head land in one [128,1024] psum tile so a
single ACTIVATE(Exp) covers them (ACT op overhead would otherwise bind).
"""

import sys

sys.path.insert(0, "/opt/trn_rl_repo")

import numpy as np
import ml_dtypes
from contextlib import ExitStack

import concourse.bass as bass
import concourse.bacc as bacc
import concourse.tile as tile
from concourse import mybir
from concourse import bass_utils

BF16 = mybir.dt.bfloat16
F32 = mybir.dt.float32
NPBF16 = ml_dtypes.bfloat16

B, T, S, E, KV, H, D = 8, 256, 4096, 512, 768, 8, 64
NC_CORES = 8


def _build_program():
    nc = bacc.Bacc("TRN2", target_bir_lowering=False, debug=False)

    ctxT_d = nc.dram_tensor("ctxT", [KV, S], BF16, kind="ExternalInput").ap()
    xT_d = nc.dram_tensor("xT", [E, T], BF16, kind="ExternalInput").ap()
    m01_d = nc.dram_tensor("m01", [128, 32], F32, kind="ExternalInput").ap()
    wqT_d = nc.dram_tensor("wqT", [E, 512], BF16, kind="ExternalInput").ap()
    wkvT_d = nc.dram_tensor("wkvT", [KV, 1024], BF16, kind="ExternalInput").ap()
    woT_d = nc.dram_tensor("woT", [512, E], BF16, kind="ExternalInput").ap()
    bo_d = nc.dram_tensor("bo_r", [128, 4], F32, kind="ExternalInput").ap()
    outT_d = nc.dram_tensor("outT", [4, 128, T], F32, kind="ExternalOutput").ap()

    ctxT_v = ctxT_d.rearrange("(c p) s -> c p s", p=128)  # [6,128,4096]
    xT_v = xT_d.rearrange("(c p) t -> c p t", p=128)  # [4,128,256]
    wqT_v = wqT_d.rearrange("(c p) m -> c p m", p=128)  # [4,128,512]
    wkvT_v = wkvT_d.rearrange("(c p) m -> c p m", p=128)  # [6,128,1024]
    woT_v = woT_d.rearrange("(c p) m -> c p m", p=128)  # [4,128,512]

    with tile.TileContext(nc) as tc, ExitStack() as ctx:
        const = ctx.enter_context(tc.tile_pool(name="const", bufs=1))
        work = ctx.enter_context(tc.tile_pool(name="work", bufs=2))
        p_pe = ctx.enter_context(tc.tile_pool(name="p_pe", bufs=3, space="PSUM"))
        p_pv = ctx.enter_context(tc.tile_pool(name="p_pv", bufs=2, space="PSUM"))

        # ---- static SBUF tensors -------------------------------------------
        # ctx quarters: ctx_t[c][q] = [128, 1024]
        ctx_t = [
            [
                const.tile([128, 1024], BF16, tag=f"ctx{c}_{q}", name=f"ctx{c}_{q}")
                for q in range(4)
            ]
            for c in range(6)
        ]
        kt_t = [
            const.tile([128, S], BF16, tag=f"kt{kc}", name=f"kt{kc}") for kc in range(4)
        ]
        vp_t = [
            const.tile([128, 8 * 65], BF16, tag=f"vp{sc}", name=f"vp{sc}")
            for sc in range(32)
        ]
        qt_t = [
            const.tile([128, T], BF16, tag=f"qt{qc}", name=f"qt{qc}") for qc in range(4)
        ]
        ot_t = [
            const.tile([128, T], BF16, tag=f"ot{cc}", name=f"ot{cc}") for cc in range(4)
        ]
        wq_t = [
            const.tile([128, 512], BF16, tag=f"wq{ec}", name=f"wq{ec}")
            for ec in range(4)
        ]
        wkv_t = [
            const.tile([128, 1024], BF16, tag=f"wkv{c}", name=f"wkv{c}")
            for c in range(6)
        ]
        wo_t = [
            const.tile([128, 512], BF16, tag=f"wo{cc}", name=f"wo{cc}")
            for cc in range(4)
        ]
        x_t = [
            const.tile([128, T], BF16, tag=f"x{ec}", name=f"x{ec}") for ec in range(4)
        ]
        pvacc_t = [
            const.tile([65, T], F32, tag=f"pvacc{h}", name=f"pvacc{h}") for h in range(8)
        ]
        den8_t = const.tile([8, T], F32, tag="den8")
        rec8_t = const.tile([8, T], F32, tag="rec8")
        rech_t = const.tile([1, 8 * T], F32, tag="rech")
        m01_t = const.tile([128, 32], F32, tag="m01")
        bo_t = const.tile([128, 4], F32, tag="bo")
        ones8_t = const.tile([128, 8], BF16, tag="ones8")
        ones64_t = const.tile([1, 64], F32, tag="ones64")

        # ---- loads ----------------------------------------------------------
        nc.vector.memset(ones8_t[:], 1.0)
        nc.vector.memset(ones64_t[:], 1.0)
        for ec in range(4):
            nc.gpsimd.dma_start(x_t[ec][:], xT_v[ec])
            nc.gpsimd.dma_start(wq_t[ec][:], wqT_v[ec])
        for c in range(6):
            nc.gpsimd.dma_start(wkv_t[c][:], wkvT_v[c])
        for c in range(3):  # first ctx quarter split across both DMA queues
            nc.sync.dma_start(ctx_t[c][0][:], ctxT_v[c][:, 0:1024])
        for c in range(3, 6):
            nc.gpsimd.dma_start(ctx_t[c][0][:], ctxT_v[c][:, 0:1024])
        for q in range(1, 4):
            for c in range(6):
                nc.sync.dma_start(
                    ctx_t[c][q][:], ctxT_v[c][:, q * 1024 : (q + 1) * 1024]
                )
        nc.gpsimd.dma_start(m01_t[:], m01_d)
        for cc in range(4):
            nc.gpsimd.dma_start(wo_t[cc][:], woT_v[cc])
        nc.gpsimd.dma_start(bo_t[:], bo_d)

        def ctx_slice(c, s0, n):
            q = s0 // 1024
            off = s0 - q * 1024
            return ctx_t[c][q][:, off : off + n]

        # ---- Q projection ---------------------------------------------------
        for qc in range(4):
            ps = p_pe.tile([128, 1024], F32, tag="pe")
            for ec in range(4):
                nc.tensor.matmul(
                    ps[:, 0:T],
                    lhsT=wq_t[ec][:, qc * 128 : (qc + 1) * 128],
                    rhs=x_t[ec][:],
                    start=(ec == 0),
                    stop=(ec == 3),
                )
            nc.vector.tensor_copy(qt_t[qc][:], ps[:, 0:T])

        # ---- interleaved KV projection + attention, per ctx quarter ---------
        # Attention group (kc, g) only needs ctx quarter g//2, so scores/exp/PV
        # for s-chunks of quarter q run right after that quarter's K/V proj.
        # PV accumulates per-quarter in PSUM, then adds into SBUF pvacc (DVE),
        # keeping only 2 PV psum banks live and the ACT exp work overlapped
        # with the next quarter's kv-proj matmuls.
        for q in range(4):
            for kc in range(4):
                ps = p_pe.tile([128, 1024], F32, tag="pe")
                for c in range(6):
                    for halfi in range(2):
                        nc.tensor.matmul(
                            ps[:, halfi * 512 : (halfi + 1) * 512],
                            lhsT=wkv_t[c][:, kc * 128 : (kc + 1) * 128],
                            rhs=ctx_slice(c, q * 1024 + halfi * 512, 512),
                            start=(c == 0),
                            stop=(c == 5),
                        )
                nc.vector.tensor_copy(
                    kt_t[kc][:, q * 1024 : (q + 1) * 1024], ps[:]
                )
            for sc in range(q * 8, (q + 1) * 8):
                ps = p_pe.tile([128, 1024], F32, tag="pe")
                for c in range(6):
                    nc.tensor.matmul(
                        ps[:, 0:512],
                        lhsT=ctx_slice(c, sc * 128, 128),
                        rhs=wkv_t[c][:, 512:1024],
                        start=(c == 0),
                        stop=(c == 5),
                    )
                dst = vp_t[sc][:].rearrange("p (h e) -> p h e", e=65)
                nc.vector.tensor_scalar_mul(
                    dst[:, :, 0:64],
                    ps[:, 0:512].rearrange("p (h d) -> p h d", d=64),
                    m01_t[:, sc : sc + 1],
                )
                nc.vector.tensor_scalar_mul(
                    dst[:, :, 64:65],
                    ones8_t[:].rearrange("p (h o) -> p h o", o=1),
                    m01_t[:, sc : sc + 1],
                )
            for kc in range(4):
                pvq0 = p_pv.tile([65, T], F32, tag="pv")
                pvq1 = p_pv.tile([65, T], F32, tag="pv")
                for g in (2 * q, 2 * q + 1):
                    pe0 = p_pe.tile([128, 1024], F32, tag="pe")
                    pe1 = p_pe.tile([128, 1024], F32, tag="pe")
                    for j in range(4):
                        sc = g * 4 + j
                        nc.tensor.matmul(
                            pe0[:, j * 256 : (j + 1) * 256],
                            lhsT=kt_t[kc][0:64, sc * 128 : (sc + 1) * 128],
                            rhs=qt_t[kc][0:64, :],
                            start=True,
                            stop=True,
                        )
                        nc.tensor.matmul(
                            pe1[:, j * 256 : (j + 1) * 256],
                            lhsT=kt_t[kc][64:128, sc * 128 : (sc + 1) * 128],
                            rhs=qt_t[kc][64:128, :],
                            start=True,
                            stop=True,
                        )
                    e0 = work.tile([128, 1024], BF16, tag="exp", bufs=6)
                    nc.scalar.activation(
                        e0[:], pe0[:], mybir.ActivationFunctionType.Exp
                    )
                    e1 = work.tile([128, 1024], BF16, tag="exp", bufs=6)
                    nc.scalar.activation(
                        e1[:], pe1[:], mybir.ActivationFunctionType.Exp
                    )
                    for j in range(4):
                        sc = g * 4 + j
                        nc.tensor.matmul(
                            pvq0[:],
                            lhsT=vp_t[sc][:, (2 * kc) * 65 : (2 * kc) * 65 + 65],
                            rhs=e0[:, j * 256 : (j + 1) * 256],
                            start=(g == 2 * q and j == 0),
                            stop=(g == 2 * q + 1 and j == 3),
                        )
                        nc.tensor.matmul(
                            pvq1[:],
                            lhsT=vp_t[sc][
                                :, (2 * kc + 1) * 65 : (2 * kc + 1) * 65 + 65
                            ],
                            rhs=e1[:, j * 256 : (j + 1) * 256],
                            start=(g == 2 * q and j == 0),
                            stop=(g == 2 * q + 1 and j == 3),
                        )
                if q == 0:
                    nc.vector.tensor_copy(pvacc_t[2 * kc][:], pvq0[:])
                    nc.vector.tensor_copy(pvacc_t[2 * kc + 1][:], pvq1[:])
                else:
                    nc.vector.tensor_add(
                        pvacc_t[2 * kc][:], pvacc_t[2 * kc][:], pvq0[:]
                    )
                    nc.vector.tensor_add(
                        pvacc_t[2 * kc + 1][:], pvacc_t[2 * kc + 1][:], pvq1[:]
                    )
                if q == 3:
                    nc.sync.dma_start(
                        den8_t[2 * kc : 2 * kc + 1, :], pvacc_t[2 * kc][64:65, :]
                    )
                    nc.gpsimd.dma_start(
                        den8_t[2 * kc + 1 : 2 * kc + 2, :],
                        pvacc_t[2 * kc + 1][64:65, :],
                    )

        # ---- deferred softmax normalization (off the PE critical path) ------
        nc.vector.reciprocal(rec8_t[:], den8_t[:])
        nc.sync.dma_start(
            rech_t[0:1, :].rearrange("p (h t) -> p h t", t=T), rec8_t[:, :]
        )
        for kc in range(4):
            bc0 = p_pv.tile([64, T], F32, tag="pv")
            nc.tensor.matmul(
                bc0[:],
                lhsT=ones64_t[:],
                rhs=rech_t[0:1, (2 * kc) * T : (2 * kc + 1) * T],
                start=True,
                stop=True,
            )
            bc1 = p_pv.tile([64, T], F32, tag="pv")
            nc.tensor.matmul(
                bc1[:],
                lhsT=ones64_t[:],
                rhs=rech_t[0:1, (2 * kc + 1) * T : (2 * kc + 2) * T],
                start=True,
                stop=True,
            )
            nc.vector.tensor_mul(ot_t[kc][0:64, :], pvacc_t[2 * kc][0:64, :], bc0[:])
            tmp1 = work.tile([64, T], BF16, tag="otmp", bufs=2)
            nc.vector.tensor_mul(tmp1[:], pvacc_t[2 * kc + 1][0:64, :], bc1[:])
            nc.sync.dma_start(ot_t[kc][64:128, :], tmp1[:])

        # ---- out projection -------------------------------------------------
        for eo in range(4):
            ps = p_pe.tile([128, 1024], F32, tag="pe")
            for cc in range(4):
                nc.tensor.matmul(
                    ps[:, 0:T],
                    lhsT=wo_t[cc][:, eo * 128 : (eo + 1) * 128],
                    rhs=ot_t[cc][:],
                    start=(cc == 0),
                    stop=(cc == 3),
                )
            osb = work.tile([128, T], F32, tag="osb", bufs=2)
            nc.vector.tensor_scalar_add(osb[:], ps[:, 0:T], bo_t[:, eo : eo + 1])
            nc.sync.dma_start(outT_d[eo], osb[:])

    nc.compile()
    return nc


_NC = None


def _get_nc():
    global _NC
    if _NC is None:
        _NC = _build_program()
    return _NC


def _prep_in_maps(x, context, key_padding_mask, Wq, Wkv, Wo, bo):
    wqT = (np.ascontiguousarray(Wq.T) * np.float32(D**-0.5)).astype(NPBF16)
    wkvT = np.ascontiguousarray(Wkv.T).astype(NPBF16)
    woT = np.ascontiguousarray(Wo.T).astype(NPBF16)
    bo_r = np.ascontiguousarray(bo.reshape(4, 128).T).astype(np.float32)
    in_maps = []
    for b in range(B):
        ctxT = np.ascontiguousarray(context[b].T).astype(NPBF16)
        xT = np.ascontiguousarray(x[b].T).astype(NPBF16)
        m01 = np.ascontiguousarray(
            (~key_padding_mask[b]).astype(np.float32).reshape(32, 128).T
        )
        in_maps.append(
            dict(ctxT=ctxT, xT=xT, m01=m01, wqT=wqT, wkvT=wkvT, woT=woT, bo_r=bo_r)
        )
    return in_maps


def _run(inputs, trace=False, **kw):
    nc = _get_nc()
    in_maps = _prep_in_maps(**inputs)
    res = bass_utils.run_bass_kernel_spmd(
        nc, in_maps, core_ids=list(range(NC_CORES)), trace=trace, **kw
    )
    out = np.stack(
        [res.results[b]["outT"].reshape(E, T).T for b in range(B)]
    ).astype(np.float32)
    return out, res


def kernel(**inputs):
    out, _ = _run(inputs, trace=False)
    return out


if __name__ == "__main__":
    rng = np.random.default_rng(0)
    ins = dict(
        x=rng.standard_normal((B, T, E), dtype=np.float32),
        context=rng.standard_normal((B, S, KV), dtype=np.float32),
        key_padding_mask=rng.integers(0, 2, (B, S)).astype(bool),
        Wq=(rng.standard_normal((512, E), dtype=np.float32) * 0.02),
        Wkv=(rng.standard_normal((1024, KV), dtype=np.float32) * 0.02),
        Wo=(rng.standard_normal((E, 512), dtype=np.float32) * 0.02),
        bo=np.zeros(E, dtype=np.float32),
    )
    out = kernel(**ins)
    print("out", out.shape, out.dtype, np.abs(out).mean())



# revision 9
# speedup vs baseline: 1.5861x; 1.5861x over previous
"""CrossAttention Trainium2 kernel (mask-compacted).

Problem (hardcoded): B=8, T=256, S=4096, E=512, KV=768, H=8, D=64.
Sharding: data-parallel over B — one batch per NeuronCore (8 cores).

Key idea vs v1: ~50% of keys are masked (key_padding_mask True = ignore)
and masked keys provably don't contribute to the output (softmax weight
exactly 0 via the m01 fold into V'). So the host compacts each batch's
context to only the kept keys, padded to a common S_pad (multiple of
128, ~2176 for the harness seed). All S-proportional device work
(KV-proj, scores, exp, PV) drops by ~1.9x. Padding rows have ctx=0 =>
k=0 => score=0 => exp=1, but m01=0 zeroes their V' rows and ones-col so
they add 0 to both numerator and denominator.

Per-core dataflow (one batch, layouts staged host-side, bf16 unless noted):
    ctxT  [768, S_pad]  = compacted context[b].T
    xT    [512, 256], wqT (scale folded), wkvT [768,1024], woT, bo_r
    m01   [128, N_SC] f32 = 1.0 kept / 0.0 pad   (s = sc*128 + p)
  device:
    QT    = wqT.T @ xT -> [512c, 256t]
    KT    = wkvT[:, :512].T @ ctxT -> [512c, S_pad]  (c-major, 4 head pairs)
    V'    = ctxT.T @ wkvT[:, 512:] -> per-sc [128 s, 8h*65] * m01
    scoresT[s,t] per head: KT head slices as lhsT (K=64, head pair packed
            into PE row groups 0:64/64:128 -> concurrent row-tiled MMs)
    expsT = Exp(scoresT) on ACT; PV = V'_h @ expsT -> [65,256], row 64 =
            softmax denominator; accumulated per 4-sc group in PSUM then
            DVE-added into SBUF pvacc.
    norm  = reciprocal_approx_fast(denoms) broadcast via K=1 matmul;
            OT = PV * recip ; outT = woT.T @ OT + bo.

Schedule: software pipeline over 512-col ctx groups g: iteration g issues
scores(g-1, kc) / kv-proj(g, kc) interleaved so the ACT exp of group g-1
runs under the kv-proj matmuls of group g, then PV(g-1, kc) / V'(g, sc)
interleaved. Tail normalization is incremental per head pair and feeds
an out-proj PSUM accumulation, keeping the PE warm to the end.
"""

import sys

sys.path.insert(0, "/opt/trn_rl_repo")

import numpy as np
import ml_dtypes
from contextlib import ExitStack

import concourse.bass as bass
import concourse.bacc as bacc
import concourse.tile as tile
from concourse import mybir
from concourse import bass_utils

BF16 = mybir.dt.bfloat16
F32 = mybir.dt.float32
NPBF16 = ml_dtypes.bfloat16

B, T, S, E, KV, H, D = 8, 256, 4096, 512, 768, 8, 64
NC_CORES = 8


def _groups(n_sc):
    """Split n_sc 128-wide s-chunks into groups of <=4 (512 ctx cols)."""
    out = []
    sc0 = 0
    while sc0 < n_sc:
        n = min(4, n_sc - sc0)
        out.append((sc0, n))
        sc0 += n
    return out


def _build_program(s_pad):
    n_sc = s_pad // 128
    groups = _groups(n_sc)
    n_g = len(groups)

    nc = bacc.Bacc("TRN2", target_bir_lowering=False, debug=False)

    ctxT_d = nc.dram_tensor("ctxT", [KV, s_pad], BF16, kind="ExternalInput").ap()
    xT_d = nc.dram_tensor("xT", [E, T], BF16, kind="ExternalInput").ap()
    m01_d = nc.dram_tensor("m01", [128, n_sc], F32, kind="ExternalInput").ap()
    wqT_d = nc.dram_tensor("wqT", [E, 512], BF16, kind="ExternalInput").ap()
    wkvT_d = nc.dram_tensor("wkvT", [KV, 1024], BF16, kind="ExternalInput").ap()
    woT_d = nc.dram_tensor("woT", [512, E], BF16, kind="ExternalInput").ap()
    bo_d = nc.dram_tensor("bo_r", [128, 4], F32, kind="ExternalInput").ap()
    outT_d = nc.dram_tensor("outT", [4, 128, T], F32, kind="ExternalOutput").ap()

    ctxT_v = ctxT_d.rearrange("(c p) s -> c p s", p=128)  # [6,128,s_pad]
    xT_v = xT_d.rearrange("(c p) t -> c p t", p=128)  # [4,128,256]
    wqT_v = wqT_d.rearrange("(c p) m -> c p m", p=128)  # [4,128,512]
    wkvT_v = wkvT_d.rearrange("(c p) m -> c p m", p=128)  # [6,128,1024]
    woT_v = woT_d.rearrange("(c p) m -> c p m", p=128)  # [4,128,512]

    with tile.TileContext(nc) as tc, ExitStack() as ctx:
        const = ctx.enter_context(tc.tile_pool(name="const", bufs=1))
        work = ctx.enter_context(tc.tile_pool(name="work", bufs=2))
        p_sc = ctx.enter_context(tc.tile_pool(name="p_sc", bufs=2, space="PSUM"))
        p_a = ctx.enter_context(tc.tile_pool(name="p_a", bufs=2, space="PSUM"))
        p_pv = ctx.enter_context(tc.tile_pool(name="p_pv", bufs=2, space="PSUM"))

        # ---- static SBUF tensors -------------------------------------------
        ctx_t = [
            [
                const.tile(
                    [128, 128 * groups[g][1]], BF16, tag=f"ctx{c}_{g}",
                    name=f"ctx{c}_{g}",
                )
                for g in range(n_g)
            ]
            for c in range(6)
        ]
        kt_t = [
            const.tile([128, s_pad], BF16, tag=f"kt{kc}", name=f"kt{kc}")
            for kc in range(4)
        ]
        vp_t = [
            const.tile([128, 8 * 65], BF16, tag=f"vp{sc}", name=f"vp{sc}")
            for sc in range(n_sc)
        ]
        qt_t = [
            const.tile([128, T], BF16, tag=f"qt{qc}", name=f"qt{qc}") for qc in range(4)
        ]
        ot_t = [
            const.tile([128, T], BF16, tag=f"ot{cc}", name=f"ot{cc}") for cc in range(4)
        ]
        wq_t = [
            const.tile([128, 512], BF16, tag=f"wq{ec}", name=f"wq{ec}")
            for ec in range(4)
        ]
        wkv_t = [
            const.tile([128, 1024], BF16, tag=f"wkv{c}", name=f"wkv{c}")
            for c in range(6)
        ]
        wo_t = [
            const.tile([128, 512], BF16, tag=f"wo{cc}", name=f"wo{cc}")
            for cc in range(4)
        ]
        x_t = [
            const.tile([128, T], BF16, tag=f"x{ec}", name=f"x{ec}") for ec in range(4)
        ]
        pvacc_t = [
            const.tile([65, T], F32, tag=f"pvacc{h}", name=f"pvacc{h}") for h in range(8)
        ]
        den2_t = [
            const.tile([2, T], F32, tag=f"den2_{kc}", name=f"den2_{kc}")
            for kc in range(4)
        ]
        rec2_t = [
            const.tile([2, T], F32, tag=f"rec2_{kc}", name=f"rec2_{kc}")
            for kc in range(4)
        ]
        rech_t = const.tile([1, 8 * T], F32, tag="rech")
        m01_t = const.tile([128, n_sc], F32, tag="m01")
        bo_t = const.tile([128, 4], F32, tag="bo")
        ones8_t = const.tile([128, 8], BF16, tag="ones8")
        ones64_t = const.tile([1, 64], F32, tag="ones64")

        # ---- loads ----------------------------------------------------------
        # 4 DMA queues for the init burst: scalar feeds Q-proj deps, gpsimd
        # the kv weights (consumed c-outer), sync the first ctx group,
        # vector the cold-path (mask, wo, bias).
        nc.vector.memset(ones8_t[:], 1.0)
        nc.vector.memset(ones64_t[:], 1.0)
        for ec in range(4):
            nc.scalar.dma_start(x_t[ec][:], xT_v[ec])
        for ec in range(4):
            nc.scalar.dma_start(wq_t[ec][:], wqT_v[ec])
        for c in range(6):
            nc.gpsimd.dma_start(wkv_t[c][:], wkvT_v[c])
        for c in range(6):
            nc.sync.dma_start(ctx_t[c][0][:], ctxT_v[c][:, 0 : 128 * groups[0][1]])
        nc.scalar.dma_start(m01_t[:], m01_d)
        nc.scalar.dma_start(bo_t[:], bo_d)
        for cc in range(4):
            nc.scalar.dma_start(wo_t[cc][:], woT_v[cc])
        # remaining ctx groups stream on sync/gpsimd alternating by c
        for g in range(1, n_g):
            sc0, nsc = groups[g]
            for c in range(6):
                q = nc.sync if c % 2 == 0 else nc.gpsimd
                q.dma_start(
                    ctx_t[c][g][:], ctxT_v[c][:, sc0 * 128 : (sc0 + nsc) * 128]
                )

        # ---- Q projection (PE warm-up while ctx/wkv stream) -----------------
        # 2 [128,1024] psum tiles; qc regions bank-aligned (one accumulation
        # group per 512-f32 bank).
        qps = [
            p_sc.tile([128, 1024], F32, tag="sc", name=f"qps{i}") for i in range(2)
        ]
        for qc in range(4):
            reg = qps[qc // 2][:, (qc % 2) * 512 : (qc % 2) * 512 + T]
            for ec in range(4):
                nc.tensor.matmul(
                    reg,
                    lhsT=wq_t[ec][:, qc * 128 : (qc + 1) * 128],
                    rhs=x_t[ec][:],
                    start=(ec == 0),
                    stop=(ec == 3),
                )
        for qc in range(4):
            nc.vector.tensor_copy(
                qt_t[qc][:], qps[qc // 2][:, (qc % 2) * 512 : (qc % 2) * 512 + T]
            )

        # ---- group-0 K-part, c-outer so PE starts on first-arrived wkv[c] --
        g0w = 128 * groups[0][1]
        aps0 = [
            p_sc.tile([128, 1024], F32, tag="sc", name=f"aps0_{i}") for i in range(2)
        ]
        for c in range(6):
            for kc in range(4):
                reg = aps0[kc // 2][:, (kc % 2) * 512 : (kc % 2) * 512 + g0w]
                nc.tensor.matmul(
                    reg,
                    lhsT=wkv_t[c][:, kc * 128 : (kc + 1) * 128],
                    rhs=ctx_t[c][0][:],
                    start=(c == 0),
                    stop=(c == 5),
                )
        for kc in range(4):
            nc.vector.tensor_copy(
                kt_t[kc][:, 0:g0w],
                aps0[kc // 2][:, (kc % 2) * 512 : (kc % 2) * 512 + g0w],
            )

        def vprime(sc):
            """V' for one 128-wide s-chunk: [128 s, 8h*65] with mask folded."""
            g = sc // 4
            off = (sc - groups[g][0]) * 128
            ps = p_a.tile([128, 512], F32, tag="a", name=f"vps{sc}")
            for c in range(6):
                nc.tensor.matmul(
                    ps[:],
                    lhsT=ctx_t[c][g][:, off : off + 128],
                    rhs=wkv_t[c][:, 512:1024],
                    start=(c == 0),
                    stop=(c == 5),
                )
            dst = vp_t[sc][:].rearrange("p (h e) -> p h e", e=65)
            nc.vector.tensor_scalar_mul(
                dst[:, :, 0:64],
                ps[:].rearrange("p (h d) -> p h d", d=64),
                m01_t[:, sc : sc + 1],
            )
            nc.vector.tensor_scalar_mul(
                dst[:, :, 64:65],
                ones8_t[:].rearrange("p (h o) -> p h o", o=1),
                m01_t[:, sc : sc + 1],
            )

        def kpart(g, kc):
            """K-projection c-major slice kc for ctx group g."""
            sc0, nsc = groups[g]
            w = nsc * 128
            ps = p_a.tile([128, 512], F32, tag="a", name=f"kps{g}_{kc}")
            for c in range(6):
                nc.tensor.matmul(
                    ps[:, 0:w],
                    lhsT=wkv_t[c][:, kc * 128 : (kc + 1) * 128],
                    rhs=ctx_t[c][g][:],
                    start=(c == 0),
                    stop=(c == 5),
                )
            nc.vector.tensor_copy(kt_t[kc][:, sc0 * 128 : sc0 * 128 + w], ps[:, 0:w])

        def scores(g, kc):
            """ScoresT + exp for head pair kc, group g. Returns (e0, e1)."""
            sc0, nsc = groups[g]
            w = nsc * 256
            pe0 = p_sc.tile([128, 1024], F32, tag="sc", name=f"pe0_{g}_{kc}")
            pe1 = p_sc.tile([128, 1024], F32, tag="sc", name=f"pe1_{g}_{kc}")
            for j in range(nsc):
                sc = sc0 + j
                nc.tensor.matmul(
                    pe0[:, j * 256 : (j + 1) * 256],
                    lhsT=kt_t[kc][0:64, sc * 128 : (sc + 1) * 128],
                    rhs=qt_t[kc][0:64, :],
                    start=True,
                    stop=True,
                )
                nc.tensor.matmul(
                    pe1[:, j * 256 : (j + 1) * 256],
                    lhsT=kt_t[kc][64:128, sc * 128 : (sc + 1) * 128],
                    rhs=qt_t[kc][64:128, :],
                    start=True,
                    stop=True,
                )
            e0 = work.tile([128, 1024], BF16, tag="exp", bufs=8, name=f"e0_{g}_{kc}")
            nc.scalar.activation(
                e0[:, 0:w], pe0[:, 0:w], mybir.ActivationFunctionType.Exp
            )
            e1 = work.tile([128, 1024], BF16, tag="exp", bufs=8, name=f"e1_{g}_{kc}")
            nc.scalar.activation(
                e1[:, 0:w], pe1[:, 0:w], mybir.ActivationFunctionType.Exp
            )
            return e0, e1

        def pv(g, kc, e0, e1):
            """PV for head pair kc over group g, accumulate into pvacc."""
            sc0, nsc = groups[g]
            pvq0 = p_pv.tile([65, T], F32, tag="pv", name=f"pvq0_{g}_{kc}")
            pvq1 = p_pv.tile([65, T], F32, tag="pv", name=f"pvq1_{g}_{kc}")
            for j in range(nsc):
                sc = sc0 + j
                nc.tensor.matmul(
                    pvq0[:],
                    lhsT=vp_t[sc][:, (2 * kc) * 65 : (2 * kc) * 65 + 65],
                    rhs=e0[:, j * 256 : (j + 1) * 256],
                    start=(j == 0),
                    stop=(j == nsc - 1),
                )
                nc.tensor.matmul(
                    pvq1[:],
                    lhsT=vp_t[sc][:, (2 * kc + 1) * 65 : (2 * kc + 1) * 65 + 65],
                    rhs=e1[:, j * 256 : (j + 1) * 256],
                    start=(j == 0),
                    stop=(j == nsc - 1),
                )
            if g == 0:
                nc.vector.tensor_copy(pvacc_t[2 * kc][:], pvq0[:])
                nc.vector.tensor_copy(pvacc_t[2 * kc + 1][:], pvq1[:])
            else:
                nc.vector.tensor_add(pvacc_t[2 * kc][:], pvacc_t[2 * kc][:], pvq0[:])
                nc.vector.tensor_add(
                    pvacc_t[2 * kc + 1][:], pvacc_t[2 * kc + 1][:], pvq1[:]
                )

        # out-proj psum: eo regions bank-aligned, accumulated over kc.
        # Allocated lazily at first tail use — allocating earlier would make
        # intermediate scores tiles alias buffers whose consumers (the tail
        # out-proj reads) come later in program order.
        ops = []

        def norm_and_outproj(kc):
            if not ops:
                ops.append(p_sc.tile([128, 1024], F32, tag="sc", name="ops0"))
                ops.append(p_sc.tile([128, 1024], F32, tag="sc", name="ops1"))
            """Normalize head pair kc and fold into the out-proj accumulation."""
            nc.sync.dma_start(den2_t[kc][0:1, :], pvacc_t[2 * kc][64:65, :])
            nc.gpsimd.dma_start(den2_t[kc][1:2, :], pvacc_t[2 * kc + 1][64:65, :])
            nc.vector.reciprocal_approx_fast(rec2_t[kc][:], den2_t[kc][:])
            nc.sync.dma_start(
                rech_t[0:1, (2 * kc) * T : (2 * kc + 2) * T].rearrange(
                    "p (h t) -> p h t", t=T
                ),
                rec2_t[kc][:],
            )
            bc = p_a.tile([128, 512], F32, tag="a", name=f"bc{kc}")
            nc.tensor.matmul(
                bc[0:64, 0:512],
                lhsT=ones64_t[:],
                rhs=rech_t[0:1, (2 * kc) * T : (2 * kc + 2) * T],
                start=True,
                stop=True,
            )
            nc.vector.tensor_mul(
                ot_t[kc][0:64, :], pvacc_t[2 * kc][0:64, :], bc[0:64, 0:T]
            )
            tmp1 = work.tile([64, T], BF16, tag="otmp", bufs=2, name=f"otmp{kc}")
            nc.vector.tensor_mul(tmp1[:], pvacc_t[2 * kc + 1][0:64, :], bc[0:64, T : 2 * T])
            nc.gpsimd.dma_start(ot_t[kc][64:128, :], tmp1[:])
            for eo in range(4):
                reg = ops[eo // 2][:, (eo % 2) * 512 : (eo % 2) * 512 + T]
                nc.tensor.matmul(
                    reg,
                    lhsT=wo_t[kc][:, eo * 128 : (eo + 1) * 128],
                    rhs=ot_t[kc][:],
                    start=(kc == 0),
                    stop=(kc == 3),
                )

        # ---- V'(0) then pipelined groups -----------------------------------
        for sc in range(groups[0][0], groups[0][0] + groups[0][1]):
            vprime(sc)

        exps = {}
        for g in range(1, n_g + 1):
            # scores(g-1) interleaved with K-part(g)
            for kc in range(4):
                exps[(g - 1, kc)] = scores(g - 1, kc)
                if g < n_g:
                    kpart(g, kc)
            # PV(g-1) interleaved with V'(g)
            for kc in range(4):
                e0, e1 = exps.pop((g - 1, kc))
                pv(g - 1, kc, e0, e1)
                if g == n_g:
                    norm_and_outproj(kc)
                elif kc < groups[g][1]:
                    vprime(groups[g][0] + kc)

        # ---- bias + output DMA ---------------------------------------------
        for eo in range(4):
            reg = ops[eo // 2][:, (eo % 2) * 512 : (eo % 2) * 512 + T]
            osb = work.tile([128, T], F32, tag="osb", bufs=2, name=f"osb{eo}")
            nc.vector.tensor_scalar_add(osb[:], reg, bo_t[:, eo : eo + 1])
            q = nc.sync if eo % 2 == 0 else nc.gpsimd
            q.dma_start(outT_d[eo], osb[:])

    nc.compile()
    return nc


_NC_CACHE = {}


def _get_nc(s_pad):
    if s_pad not in _NC_CACHE:
        _NC_CACHE[s_pad] = _build_program(s_pad)
    return _NC_CACHE[s_pad]


def _prep_in_maps(x, context, key_padding_mask, Wq, Wkv, Wo, bo):
    keep = [np.flatnonzero(~key_padding_mask[b]) for b in range(B)]
    max_keep = max(len(k) for k in keep)
    s_pad = max(128, -(-max_keep // 128) * 128)

    wqT = (np.ascontiguousarray(Wq.T) * np.float32(D**-0.5)).astype(NPBF16)
    wkvT = np.ascontiguousarray(Wkv.T).astype(NPBF16)
    woT = np.ascontiguousarray(Wo.T).astype(NPBF16)
    bo_r = np.ascontiguousarray(bo.reshape(4, 128).T).astype(np.float32)
    in_maps = []
    for b in range(B):
        nk = len(keep[b])
        ctxc = np.zeros((s_pad, KV), dtype=np.float32)
        ctxc[:nk] = context[b][keep[b]]
        ctxT = np.ascontiguousarray(ctxc.T).astype(NPBF16)
        xT = np.ascontiguousarray(x[b].T).astype(NPBF16)
        m = np.zeros(s_pad, dtype=np.float32)
        m[:nk] = 1.0
        m01 = np.ascontiguousarray(m.reshape(s_pad // 128, 128).T)
        in_maps.append(
            dict(ctxT=ctxT, xT=xT, m01=m01, wqT=wqT, wkvT=wkvT, woT=woT, bo_r=bo_r)
        )
    return in_maps, s_pad


def _run(inputs, trace=False, **kw):
    in_maps, s_pad = _prep_in_maps(**inputs)
    nc = _get_nc(s_pad)
    res = bass_utils.run_bass_kernel_spmd(
        nc, in_maps, core_ids=list(range(NC_CORES)), trace=trace, **kw
    )
    out = np.stack(
        [res.results[b]["outT"].reshape(E, T).T for b in range(B)]
    ).astype(np.float32)
    return out, res


def kernel(**inputs):
    out, _ = _run(inputs, trace=False)
    return out


if __name__ == "__main__":
    rng = np.random.default_rng(0)
    ins = dict(
        x=rng.standard_normal((B, T, E), dtype=np.float32),
        context=rng.standard_normal((B, S, KV), dtype=np.float32),
        key_padding_mask=rng.integers(0, 2, (B, S)).astype(bool),
        Wq=(rng.standard_normal((512, E), dtype=np.float32) * 0.02),
        Wkv=(rng.standard_normal((1024, KV), dtype=np.float32) * 0.02),
        Wo=(rng.standard_normal((E, 512), dtype=np.float32) * 0.02),
        bo=np.zeros(E, dtype=np.float32),
    )
    out = kernel(**ins)
    print("out", out.shape, out.dtype, np.abs(out).mean())


# revision 14
# speedup vs baseline: 1.6102x; 1.0152x over previous
"""CrossAttention Trainium2 kernel (mask-compacted).

Problem (hardcoded): B=8, T=256, S=4096, E=512, KV=768, H=8, D=64.
Sharding: data-parallel over B — one batch per NeuronCore (8 cores).

Key idea vs v1: ~50% of keys are masked (key_padding_mask True = ignore)
and masked keys provably don't contribute to the output (softmax weight
exactly 0 via the m01 fold into V'). So the host compacts each batch's
context to only the kept keys, padded to a common S_pad (multiple of
128, ~2176 for the harness seed). All S-proportional device work
(KV-proj, scores, exp, PV) drops by ~1.9x. Padding rows have ctx=0 =>
k=0 => score=0 => exp=1, but m01=0 zeroes their V' rows and ones-col so
they add 0 to both numerator and denominator.

Per-core dataflow (one batch, layouts staged host-side, bf16 unless noted):
    ctxT  [768, S_pad]  = compacted context[b].T
    xT    [512, 256], wqT (scale folded), wkvT [768,1024], woT, bo_r
    m01   [128, N_SC] f32 = 1.0 kept / 0.0 pad   (s = sc*128 + p)
  device:
    QT    = wqT.T @ xT -> [512c, 256t]
    KT    = wkvT[:, :512].T @ ctxT -> [512c, S_pad]  (c-major, 4 head pairs)
    V'    = ctxT.T @ wkvT[:, 512:] -> per-sc [128 s, 8h*65] * m01
    scoresT[s,t] per head: KT head slices as lhsT (K=64, head pair packed
            into PE row groups 0:64/64:128 -> concurrent row-tiled MMs)
    expsT = Exp(scoresT) on ACT; PV = V'_h @ expsT -> [65,256], row 64 =
            softmax denominator; accumulated per 4-sc group in PSUM then
            DVE-added into SBUF pvacc.
    norm  = reciprocal_approx_fast(denoms) broadcast via K=1 matmul;
            OT = PV * recip ; outT = woT.T @ OT + bo.

Schedule: software pipeline over 512-col ctx groups g: iteration g issues
scores(g-1, kc) / kv-proj(g, kc) interleaved so the ACT exp of group g-1
runs under the kv-proj matmuls of group g, then PV(g-1, kc) / V'(g, sc)
interleaved. Tail normalization is incremental per head pair and feeds
an out-proj PSUM accumulation, keeping the PE warm to the end.
"""

import sys

sys.path.insert(0, "/opt/trn_rl_repo")

import numpy as np
import ml_dtypes
from contextlib import ExitStack

import concourse.bass as bass
import concourse.bacc as bacc
import concourse.tile as tile
from concourse import mybir
from concourse import bass_utils

BF16 = mybir.dt.bfloat16
F32 = mybir.dt.float32
NPBF16 = ml_dtypes.bfloat16

B, T, S, E, KV, H, D = 8, 256, 4096, 512, 768, 8, 64
NC_CORES = 8


def _groups(n_sc):
    """Split n_sc 128-wide s-chunks into groups of <=4 (512 ctx cols).

    The remainder group goes FIRST: a tiny group 0 lets the PE start on
    kv-proj as soon as possible (small first ctx DMA), and a full-size
    last group gives the software pipeline real PE work to overlap the
    tail normalization latency with.
    """
    rem = n_sc % 4
    out = []
    sc0 = 0
    if rem:
        out.append((0, rem))
        sc0 = rem
    while sc0 < n_sc:
        out.append((sc0, 4))
        sc0 += 4
    return out


def _build_program(s_pad):
    n_sc = s_pad // 128
    groups = _groups(n_sc)
    n_g = len(groups)

    nc = bacc.Bacc("TRN2", target_bir_lowering=False, debug=False)

    ctxT_d = nc.dram_tensor("ctxT", [KV, s_pad], BF16, kind="ExternalInput").ap()
    xT_d = nc.dram_tensor("xT", [E, T], BF16, kind="ExternalInput").ap()
    m01_d = nc.dram_tensor("m01", [128, n_sc], F32, kind="ExternalInput").ap()
    wqT_d = nc.dram_tensor("wqT", [E, 512], BF16, kind="ExternalInput").ap()
    wkvT_d = nc.dram_tensor("wkvT", [KV, 1024], BF16, kind="ExternalInput").ap()
    woT_d = nc.dram_tensor("woT", [512, E], BF16, kind="ExternalInput").ap()
    bo_d = nc.dram_tensor("bo_r", [128, 4], F32, kind="ExternalInput").ap()
    outT_d = nc.dram_tensor("outT", [4, 128, T], F32, kind="ExternalOutput").ap()

    ctxT_v = ctxT_d.rearrange("(c p) s -> c p s", p=128)  # [6,128,s_pad]
    xT_v = xT_d.rearrange("(c p) t -> c p t", p=128)  # [4,128,256]
    wqT_v = wqT_d.rearrange("(c p) m -> c p m", p=128)  # [4,128,512]
    wkvT_v = wkvT_d.rearrange("(c p) m -> c p m", p=128)  # [6,128,1024]
    woT_v = woT_d.rearrange("(c p) m -> c p m", p=128)  # [4,128,512]

    with tile.TileContext(nc) as tc, ExitStack() as ctx:
        const = ctx.enter_context(tc.tile_pool(name="const", bufs=1))
        work = ctx.enter_context(tc.tile_pool(name="work", bufs=2))
        p_sc = ctx.enter_context(tc.tile_pool(name="p_sc", bufs=2, space="PSUM"))
        p_a = ctx.enter_context(tc.tile_pool(name="p_a", bufs=2, space="PSUM"))
        p_pv = ctx.enter_context(tc.tile_pool(name="p_pv", bufs=2, space="PSUM"))

        # ---- static SBUF tensors -------------------------------------------
        ctx_t = [
            [
                const.tile(
                    [128, 128 * groups[g][1]], BF16, tag=f"ctx{c}_{g}",
                    name=f"ctx{c}_{g}",
                )
                for g in range(n_g)
            ]
            for c in range(6)
        ]
        kt_t = [
            const.tile([128, s_pad], BF16, tag=f"kt{kc}", name=f"kt{kc}")
            for kc in range(4)
        ]
        vp_t = [
            const.tile([128, 8 * 65], BF16, tag=f"vp{sc}", name=f"vp{sc}")
            for sc in range(n_sc)
        ]
        qt_t = [
            const.tile([128, T], BF16, tag=f"qt{qc}", name=f"qt{qc}") for qc in range(4)
        ]
        ot_t = [
            const.tile([128, T], BF16, tag=f"ot{cc}", name=f"ot{cc}") for cc in range(4)
        ]
        wq_t = [
            const.tile([128, 512], BF16, tag=f"wq{ec}", name=f"wq{ec}")
            for ec in range(4)
        ]
        wkv_t = [
            const.tile([128, 1024], BF16, tag=f"wkv{c}", name=f"wkv{c}")
            for c in range(6)
        ]
        wo_t = [
            const.tile([128, 512], BF16, tag=f"wo{cc}", name=f"wo{cc}")
            for cc in range(4)
        ]
        x_t = [
            const.tile([128, T], BF16, tag=f"x{ec}", name=f"x{ec}") for ec in range(4)
        ]
        pvacc_t = [
            const.tile([65, T], F32, tag=f"pvacc{h}", name=f"pvacc{h}") for h in range(8)
        ]
        den2_t = [
            const.tile([2, T], F32, tag=f"den2_{kc}", name=f"den2_{kc}")
            for kc in range(4)
        ]
        rec2_t = [
            const.tile([2, T], F32, tag=f"rec2_{kc}", name=f"rec2_{kc}")
            for kc in range(4)
        ]
        rech_t = const.tile([1, 8 * T], F32, tag="rech")
        m01_t = const.tile([128, n_sc], F32, tag="m01")
        bo_t = const.tile([128, 4], F32, tag="bo")
        ones8_t = const.tile([128, 8], BF16, tag="ones8")
        ones64_t = const.tile([1, 64], F32, tag="ones64")

        # ---- loads ----------------------------------------------------------
        # 3 DMA queues (sync/SP, gpsimd, scalar/Activation). scalar feeds the
        # Q-proj deps; the first (tiny) ctx group and the kv weights split
        # across sync+gpsimd so kv-proj group 0 can start earliest.
        nc.vector.memset(ones8_t[:], 1.0)
        nc.vector.memset(ones64_t[:], 1.0)
        g0w = 128 * groups[0][1]
        for c in range(6):
            q = nc.sync if c % 2 == 0 else nc.gpsimd
            q.dma_start(ctx_t[c][0][:], ctxT_v[c][:, 0:g0w])
        for ec in range(4):
            nc.scalar.dma_start(x_t[ec][:], xT_v[ec])
        for ec in range(4):
            nc.scalar.dma_start(wq_t[ec][:], wqT_v[ec])
        for c in range(6):
            q = nc.sync if c % 2 == 0 else nc.gpsimd
            q.dma_start(wkv_t[c][:], wkvT_v[c])
        nc.scalar.dma_start(m01_t[:], m01_d)
        nc.scalar.dma_start(bo_t[:], bo_d)
        for cc in range(4):
            nc.scalar.dma_start(wo_t[cc][:], woT_v[cc])
        # remaining ctx groups stream on sync/gpsimd alternating by c
        for g in range(1, n_g):
            sc0, nsc = groups[g]
            for c in range(6):
                q = nc.sync if c % 2 == 0 else nc.gpsimd
                q.dma_start(
                    ctx_t[c][g][:], ctxT_v[c][:, sc0 * 128 : (sc0 + nsc) * 128]
                )

        # ---- Q projection (PE warm-up while ctx/wkv stream) -----------------
        # 2 [128,1024] psum tiles; qc regions bank-aligned (one accumulation
        # group per 512-f32 bank).
        qps = [
            p_sc.tile([128, 1024], F32, tag="sc", name=f"qps{i}") for i in range(2)
        ]
        for qc in range(4):
            reg = qps[qc // 2][:, (qc % 2) * 512 : (qc % 2) * 512 + T]
            for ec in range(4):
                nc.tensor.matmul(
                    reg,
                    lhsT=wq_t[ec][:, qc * 128 : (qc + 1) * 128],
                    rhs=x_t[ec][:],
                    start=(ec == 0),
                    stop=(ec == 3),
                )
        for qc in range(4):
            nc.vector.tensor_copy(
                qt_t[qc][:], qps[qc // 2][:, (qc % 2) * 512 : (qc % 2) * 512 + T]
            )

        # ---- group-0 K-part, c-outer so PE starts on first-arrived wkv[c] --
        aps0 = [
            p_sc.tile([128, 1024], F32, tag="sc", name=f"aps0_{i}") for i in range(2)
        ]
        for c in range(6):
            for kc in range(4):
                reg = aps0[kc // 2][:, (kc % 2) * 512 : (kc % 2) * 512 + g0w]
                nc.tensor.matmul(
                    reg,
                    lhsT=wkv_t[c][:, kc * 128 : (kc + 1) * 128],
                    rhs=ctx_t[c][0][:],
                    start=(c == 0),
                    stop=(c == 5),
                )
        for kc in range(4):
            nc.vector.tensor_copy(
                kt_t[kc][:, 0:g0w],
                aps0[kc // 2][:, (kc % 2) * 512 : (kc % 2) * 512 + g0w],
            )

        sc2g = {}
        for gi, (sc0_, nsc_) in enumerate(groups):
            for sc_ in range(sc0_, sc0_ + nsc_):
                sc2g[sc_] = gi

        def vprime(sc):
            """V' for one 128-wide s-chunk: [128 s, 8h*65] with mask folded."""
            g = sc2g[sc]
            off = (sc - groups[g][0]) * 128
            ps = p_a.tile([128, 512], F32, tag="a", name=f"vps{sc}")
            for c in range(6):
                nc.tensor.matmul(
                    ps[:],
                    lhsT=ctx_t[c][g][:, off : off + 128],
                    rhs=wkv_t[c][:, 512:1024],
                    start=(c == 0),
                    stop=(c == 5),
                )
            dst = vp_t[sc][:].rearrange("p (h e) -> p h e", e=65)
            nc.vector.tensor_scalar_mul(
                dst[:, :, 0:64],
                ps[:].rearrange("p (h d) -> p h d", d=64),
                m01_t[:, sc : sc + 1],
            )
            nc.vector.tensor_scalar_mul(
                dst[:, :, 64:65],
                ones8_t[:].rearrange("p (h o) -> p h o", o=1),
                m01_t[:, sc : sc + 1],
            )

        def kpart(g, kc):
            """K-projection c-major slice kc for ctx group g."""
            sc0, nsc = groups[g]
            w = nsc * 128
            ps = p_a.tile([128, 512], F32, tag="a", name=f"kps{g}_{kc}")
            for c in range(6):
                nc.tensor.matmul(
                    ps[:, 0:w],
                    lhsT=wkv_t[c][:, kc * 128 : (kc + 1) * 128],
                    rhs=ctx_t[c][g][:],
                    start=(c == 0),
                    stop=(c == 5),
                )
            nc.vector.tensor_copy(kt_t[kc][:, sc0 * 128 : sc0 * 128 + w], ps[:, 0:w])

        def scores(g, kc):
            """ScoresT + exp for head pair kc, group g. Returns (e0, e1)."""
            sc0, nsc = groups[g]
            w = nsc * 256
            pe0 = p_sc.tile([128, 1024], F32, tag="sc", name=f"pe0_{g}_{kc}")
            pe1 = p_sc.tile([128, 1024], F32, tag="sc", name=f"pe1_{g}_{kc}")
            for j in range(nsc):
                sc = sc0 + j
                nc.tensor.matmul(
                    pe0[:, j * 256 : (j + 1) * 256],
                    lhsT=kt_t[kc][0:64, sc * 128 : (sc + 1) * 128],
                    rhs=qt_t[kc][0:64, :],
                    start=True,
                    stop=True,
                )
                nc.tensor.matmul(
                    pe1[:, j * 256 : (j + 1) * 256],
                    lhsT=kt_t[kc][64:128, sc * 128 : (sc + 1) * 128],
                    rhs=qt_t[kc][64:128, :],
                    start=True,
                    stop=True,
                )
            e0 = work.tile([128, 1024], BF16, tag="exp", bufs=8, name=f"e0_{g}_{kc}")
            nc.scalar.activation(
                e0[:, 0:w], pe0[:, 0:w], mybir.ActivationFunctionType.Exp
            )
            e1 = work.tile([128, 1024], BF16, tag="exp", bufs=8, name=f"e1_{g}_{kc}")
            nc.scalar.activation(
                e1[:, 0:w], pe1[:, 0:w], mybir.ActivationFunctionType.Exp
            )
            return e0, e1

        def pv(g, kc, e0, e1):
            """PV for head pair kc over group g, accumulate into pvacc."""
            sc0, nsc = groups[g]
            pvq0 = p_pv.tile([65, T], F32, tag="pv", name=f"pvq0_{g}_{kc}")
            pvq1 = p_pv.tile([65, T], F32, tag="pv", name=f"pvq1_{g}_{kc}")
            for j in range(nsc):
                sc = sc0 + j
                nc.tensor.matmul(
                    pvq0[:],
                    lhsT=vp_t[sc][:, (2 * kc) * 65 : (2 * kc) * 65 + 65],
                    rhs=e0[:, j * 256 : (j + 1) * 256],
                    start=(j == 0),
                    stop=(j == nsc - 1),
                )
                nc.tensor.matmul(
                    pvq1[:],
                    lhsT=vp_t[sc][:, (2 * kc + 1) * 65 : (2 * kc + 1) * 65 + 65],
                    rhs=e1[:, j * 256 : (j + 1) * 256],
                    start=(j == 0),
                    stop=(j == nsc - 1),
                )
            if g == 0:
                nc.vector.tensor_copy(pvacc_t[2 * kc][:], pvq0[:])
                nc.vector.tensor_copy(pvacc_t[2 * kc + 1][:], pvq1[:])
            else:
                nc.vector.tensor_add(pvacc_t[2 * kc][:], pvacc_t[2 * kc][:], pvq0[:])
                nc.vector.tensor_add(
                    pvacc_t[2 * kc + 1][:], pvacc_t[2 * kc + 1][:], pvq1[:]
                )

        # out-proj psum: eo regions bank-aligned, accumulated over kc.
        # Allocated lazily at first tail use — allocating earlier would make
        # intermediate scores tiles alias buffers whose consumers (the tail
        # out-proj reads) come later in program order.
        ops = []

        def den_chain(kc):
            """Collect denominators of head pair kc, reciprocal, refold to rech."""
            nc.sync.dma_start(den2_t[kc][0:1, :], pvacc_t[2 * kc][64:65, :])
            nc.gpsimd.dma_start(den2_t[kc][1:2, :], pvacc_t[2 * kc + 1][64:65, :])
            nc.vector.reciprocal_approx_fast(rec2_t[kc][:], den2_t[kc][:])
            nc.sync.dma_start(
                rech_t[0:1, (2 * kc) * T : (2 * kc + 2) * T].rearrange(
                    "p (h t) -> p h t", t=T
                ),
                rec2_t[kc][:],
            )

        def bc_mul(kc):
            """Broadcast 1/den over 64 partitions (K=1 matmul), scale PV -> OT."""
            bc = p_a.tile([128, 512], F32, tag="a", name=f"bc{kc}")
            nc.tensor.matmul(
                bc[0:64, 0:512],
                lhsT=ones64_t[:],
                rhs=rech_t[0:1, (2 * kc) * T : (2 * kc + 2) * T],
                start=True,
                stop=True,
            )
            tmp1 = work.tile([64, T], BF16, tag="otmp", bufs=2, name=f"otmp{kc}")
            nc.vector.tensor_mul(
                tmp1[:], pvacc_t[2 * kc + 1][0:64, :], bc[0:64, T : 2 * T]
            )
            nc.gpsimd.dma_start(ot_t[kc][64:128, :], tmp1[:])
            nc.vector.tensor_mul(
                ot_t[kc][0:64, :], pvacc_t[2 * kc][0:64, :], bc[0:64, 0:T]
            )

        def outproj(kc):
            if not ops:
                ops.append(p_sc.tile([128, 1024], F32, tag="sc", name="ops0"))
                ops.append(p_sc.tile([128, 1024], F32, tag="sc", name="ops1"))
            for eo in range(4):
                reg = ops[eo // 2][:, (eo % 2) * 512 : (eo % 2) * 512 + T]
                nc.tensor.matmul(
                    reg,
                    lhsT=wo_t[kc][:, eo * 128 : (eo + 1) * 128],
                    rhs=ot_t[kc][:],
                    start=(kc == 0),
                    stop=(kc == 3),
                )

        # ---- V'(0) then pipelined groups -----------------------------------
        for sc in range(groups[0][0], groups[0][0] + groups[0][1]):
            vprime(sc)

        exps = {}
        for g in range(1, n_g):
            # scores(g-1) interleaved with K-part(g)
            for kc in range(4):
                exps[(g - 1, kc)] = scores(g - 1, kc)
                kpart(g, kc)
            # PV(g-1) interleaved with V'(g)
            for kc in range(4):
                e0, e1 = exps.pop((g - 1, kc))
                pv(g - 1, kc, e0, e1)
                if kc < groups[g][1]:
                    vprime(groups[g][0] + kc)

        # ---- final group: scores/PV interleaved with the normalization tail.
        # PV(kc)+den_chain(kc) slot between scores issues so the PE never
        # stalls on the den->recip->rech DMA latency; bc/outproj follow once
        # their reciprocals are in flight.
        gl = n_g - 1
        for kc in range(4):
            exps[(gl, kc)] = scores(gl, kc)
            if kc >= 1:
                e0, e1 = exps.pop((gl, kc - 1))
                pv(gl, kc - 1, e0, e1)
                den_chain(kc - 1)
        e0, e1 = exps.pop((gl, 3))
        pv(gl, 3, e0, e1)
        den_chain(3)
        for kc in range(4):
            bc_mul(kc)
        for kc in range(4):
            outproj(kc)

        # ---- bias + output DMA ---------------------------------------------
        for eo in range(4):
            reg = ops[eo // 2][:, (eo % 2) * 512 : (eo % 2) * 512 + T]
            osb = work.tile([128, T], F32, tag="osb", bufs=4, name=f"osb{eo}")
            nc.vector.tensor_scalar_add(osb[:], reg, bo_t[:, eo : eo + 1])
            q = nc.sync if eo % 2 == 0 else nc.gpsimd
            q.dma_start(outT_d[eo], osb[:])

    nc.compile()
    return nc


_NC_CACHE = {}


def _get_nc(s_pad):
    if s_pad not in _NC_CACHE:
        _NC_CACHE[s_pad] = _build_program(s_pad)
    return _NC_CACHE[s_pad]


def _prep_in_maps(x, context, key_padding_mask, Wq, Wkv, Wo, bo):
    keep = [np.flatnonzero(~key_padding_mask[b]) for b in range(B)]
    max_keep = max(len(k) for k in keep)
    s_pad = max(128, -(-max_keep // 128) * 128)

    wqT = (np.ascontiguousarray(Wq.T) * np.float32(D**-0.5)).astype(NPBF16)
    wkvT = np.ascontiguousarray(Wkv.T).astype(NPBF16)
    woT = np.ascontiguousarray(Wo.T).astype(NPBF16)
    bo_r = np.ascontiguousarray(bo.reshape(4, 128).T).astype(np.float32)
    in_maps = []
    for b in range(B):
        nk = len(keep[b])
        ctxc = np.zeros((s_pad, KV), dtype=np.float32)
        ctxc[:nk] = context[b][keep[b]]
        ctxT = np.ascontiguousarray(ctxc.T).astype(NPBF16)
        xT = np.ascontiguousarray(x[b].T).astype(NPBF16)
        m = np.zeros(s_pad, dtype=np.float32)
        m[:nk] = 1.0
        m01 = np.ascontiguousarray(m.reshape(s_pad // 128, 128).T)
        in_maps.append(
            dict(ctxT=ctxT, xT=xT, m01=m01, wqT=wqT, wkvT=wkvT, woT=woT, bo_r=bo_r)
        )
    return in_maps, s_pad


def _run(inputs, trace=False, **kw):
    in_maps, s_pad = _prep_in_maps(**inputs)
    nc = _get_nc(s_pad)
    res = bass_utils.run_bass_kernel_spmd(
        nc, in_maps, core_ids=list(range(NC_CORES)), trace=trace, **kw
    )
    out = np.stack(
        [res.results[b]["outT"].reshape(E, T).T for b in range(B)]
    ).astype(np.float32)
    return out, res


def kernel(**inputs):
    out, _ = _run(inputs, trace=False)
    return out


if __name__ == "__main__":
    rng = np.random.default_rng(0)
    ins = dict(
        x=rng.standard_normal((B, T, E), dtype=np.float32),
        context=rng.standard_normal((B, S, KV), dtype=np.float32),
        key_padding_mask=rng.integers(0, 2, (B, S)).astype(bool),
        Wq=(rng.standard_normal((512, E), dtype=np.float32) * 0.02),
        Wkv=(rng.standard_normal((1024, KV), dtype=np.float32) * 0.02),
        Wo=(rng.standard_normal((E, 512), dtype=np.float32) * 0.02),
        bo=np.zeros(E, dtype=np.float32),
    )
    out = kernel(**ins)
    print("out", out.shape, out.dtype, np.abs(out).mean())


# revision 16
# speedup vs baseline: 1.6155x; 1.0033x over previous
"""CrossAttention Trainium2 kernel (mask-compacted).

Problem (hardcoded): B=8, T=256, S=4096, E=512, KV=768, H=8, D=64.
Sharding: data-parallel over B — one batch per NeuronCore (8 cores).

Key idea vs v1: ~50% of keys are masked (key_padding_mask True = ignore)
and masked keys provably don't contribute to the output (softmax weight
exactly 0 via the m01 fold into V'). So the host compacts each batch's
context to only the kept keys, padded to a common S_pad (multiple of
128, ~2176 for the harness seed). All S-proportional device work
(KV-proj, scores, exp, PV) drops by ~1.9x. Padding rows have ctx=0 =>
k=0 => score=0 => exp=1, but m01=0 zeroes their V' rows and ones-col so
they add 0 to both numerator and denominator.

Per-core dataflow (one batch, layouts staged host-side, bf16 unless noted):
    ctxT  [768, S_pad]  = compacted context[b].T
    xT    [512, 256], wqT (scale folded), wkvT [768,1024], woT, bo_r
    m01   [128, N_SC] f32 = 1.0 kept / 0.0 pad   (s = sc*128 + p)
  device:
    QT    = wqT.T @ xT -> [512c, 256t]
    KT    = wkvT[:, :512].T @ ctxT -> [512c, S_pad]  (c-major, 4 head pairs)
    V'    = ctxT.T @ wkvT[:, 512:] -> per-sc [128 s, 8h*65] * m01
    scoresT[s,t] per head: KT head slices as lhsT (K=64, head pair packed
            into PE row groups 0:64/64:128 -> concurrent row-tiled MMs)
    expsT = Exp(scoresT) on ACT; PV = V'_h @ expsT -> [65,256], row 64 =
            softmax denominator; accumulated per 4-sc group in PSUM then
            DVE-added into SBUF pvacc.
    norm  = reciprocal_approx_fast(denoms) broadcast via K=1 matmul;
            OT = PV * recip ; outT = woT.T @ OT + bo.

Schedule: software pipeline over 512-col ctx groups g: iteration g issues
scores(g-1, kc) / kv-proj(g, kc) interleaved so the ACT exp of group g-1
runs under the kv-proj matmuls of group g, then PV(g-1, kc) / V'(g, sc)
interleaved. Tail normalization is incremental per head pair and feeds
an out-proj PSUM accumulation, keeping the PE warm to the end.
"""

import sys

sys.path.insert(0, "/opt/trn_rl_repo")

import numpy as np
import ml_dtypes
from contextlib import ExitStack

import concourse.bass as bass
import concourse.bacc as bacc
import concourse.tile as tile
from concourse import mybir
from concourse import bass_utils

BF16 = mybir.dt.bfloat16
F32 = mybir.dt.float32
NPBF16 = ml_dtypes.bfloat16

B, T, S, E, KV, H, D = 8, 256, 4096, 512, 768, 8, 64
NC_CORES = 8


def _groups(n_sc):
    """Split n_sc 128-wide s-chunks into groups of <=4 (512 ctx cols).

    The remainder group goes FIRST: a tiny group 0 lets the PE start on
    kv-proj as soon as possible (small first ctx DMA), and a full-size
    last group gives the software pipeline real PE work to overlap the
    tail normalization latency with.
    """
    rem = n_sc % 4
    out = []
    sc0 = 0
    if rem:
        out.append((0, rem))
        sc0 = rem
    while sc0 < n_sc:
        out.append((sc0, 4))
        sc0 += 4
    return out


def _build_program(s_pad):
    n_sc = s_pad // 128
    groups = _groups(n_sc)
    n_g = len(groups)

    nc = bacc.Bacc("TRN2", target_bir_lowering=False, debug=False)

    ctxT_d = nc.dram_tensor("ctxT", [KV, s_pad], BF16, kind="ExternalInput").ap()
    xT_d = nc.dram_tensor("xT", [E, T], BF16, kind="ExternalInput").ap()
    m01_d = nc.dram_tensor("m01", [128, n_sc], F32, kind="ExternalInput").ap()
    wqT_d = nc.dram_tensor("wqT", [E, 512], BF16, kind="ExternalInput").ap()
    wkvT_d = nc.dram_tensor("wkvT", [KV, 1024], BF16, kind="ExternalInput").ap()
    woT_d = nc.dram_tensor("woT", [512, E], BF16, kind="ExternalInput").ap()
    bo_d = nc.dram_tensor("bo_r", [128, 4], F32, kind="ExternalInput").ap()
    outT_d = nc.dram_tensor("outT", [4, 128, T], F32, kind="ExternalOutput").ap()

    ctxT_v = ctxT_d.rearrange("(c p) s -> c p s", p=128)  # [6,128,s_pad]
    xT_v = xT_d.rearrange("(c p) t -> c p t", p=128)  # [4,128,256]
    wqT_v = wqT_d.rearrange("(c p) m -> c p m", p=128)  # [4,128,512]
    wkvT_v = wkvT_d.rearrange("(c p) m -> c p m", p=128)  # [6,128,1024]
    woT_v = woT_d.rearrange("(c p) m -> c p m", p=128)  # [4,128,512]

    with tile.TileContext(nc) as tc, ExitStack() as ctx:
        const = ctx.enter_context(tc.tile_pool(name="const", bufs=1))
        work = ctx.enter_context(tc.tile_pool(name="work", bufs=2))
        p_sc = ctx.enter_context(tc.tile_pool(name="p_sc", bufs=2, space="PSUM"))
        p_a = ctx.enter_context(tc.tile_pool(name="p_a", bufs=2, space="PSUM"))
        p_pv = ctx.enter_context(tc.tile_pool(name="p_pv", bufs=2, space="PSUM"))

        # ---- static SBUF tensors -------------------------------------------
        ctx_t = [
            [
                const.tile(
                    [128, 128 * groups[g][1]], BF16, tag=f"ctx{c}_{g}",
                    name=f"ctx{c}_{g}",
                )
                for g in range(n_g)
            ]
            for c in range(6)
        ]
        kt_t = [
            const.tile([128, s_pad], BF16, tag=f"kt{kc}", name=f"kt{kc}")
            for kc in range(4)
        ]
        vp_t = [
            const.tile([128, 8 * 65], BF16, tag=f"vp{sc}", name=f"vp{sc}")
            for sc in range(n_sc)
        ]
        qt_t = [
            const.tile([128, T], BF16, tag=f"qt{qc}", name=f"qt{qc}") for qc in range(4)
        ]
        ot_t = [
            const.tile([128, T], BF16, tag=f"ot{cc}", name=f"ot{cc}") for cc in range(4)
        ]
        wq_t = [
            const.tile([128, 512], BF16, tag=f"wq{ec}", name=f"wq{ec}")
            for ec in range(4)
        ]
        wkv_t = [
            const.tile([128, 1024], BF16, tag=f"wkv{c}", name=f"wkv{c}")
            for c in range(6)
        ]
        wo_t = [
            const.tile([128, 512], BF16, tag=f"wo{cc}", name=f"wo{cc}")
            for cc in range(4)
        ]
        x_t = [
            const.tile([128, T], BF16, tag=f"x{ec}", name=f"x{ec}") for ec in range(4)
        ]
        pvacc_t = [
            const.tile([65, T], F32, tag=f"pvacc{h}", name=f"pvacc{h}") for h in range(8)
        ]
        den2_t = [
            const.tile([2, T], F32, tag=f"den2_{kc}", name=f"den2_{kc}")
            for kc in range(4)
        ]
        rec2_t = [
            const.tile([2, T], F32, tag=f"rec2_{kc}", name=f"rec2_{kc}")
            for kc in range(4)
        ]
        rech_t = const.tile([1, 8 * T], F32, tag="rech")
        m01_t = const.tile([128, n_sc], F32, tag="m01")
        bo_t = const.tile([128, 4], F32, tag="bo")
        ones8_t = const.tile([128, 8], BF16, tag="ones8")
        ones64_t = const.tile([1, 64], F32, tag="ones64")

        # ---- loads ----------------------------------------------------------
        # 3 DMA queues (sync/SP, gpsimd, scalar/Activation).
        #   sync:   x, wq (Q-proj deps) then later ctx groups
        #   gpsimd: ctx group 0 (tiny) + group 1 then later groups
        #   scalar: wkv (consumed c-outer by group-0 K-part), then cold path
        nc.vector.memset(ones8_t[:], 1.0)
        nc.vector.memset(ones64_t[:], 1.0)
        g0w = 128 * groups[0][1]
        for ec in range(4):
            nc.sync.dma_start(x_t[ec][:], xT_v[ec])
        for c in range(6):
            nc.gpsimd.dma_start(ctx_t[c][0][:], ctxT_v[c][:, 0:g0w])
        for ec in range(4):
            nc.sync.dma_start(wq_t[ec][:], wqT_v[ec])
        for c in range(6):
            nc.scalar.dma_start(wkv_t[c][:], wkvT_v[c])
        sc1, nsc1 = groups[1]
        for c in range(6):
            nc.gpsimd.dma_start(
                ctx_t[c][1][:], ctxT_v[c][:, sc1 * 128 : (sc1 + nsc1) * 128]
            )
        nc.scalar.dma_start(m01_t[:], m01_d)
        nc.scalar.dma_start(bo_t[:], bo_d)
        for cc in range(4):
            nc.scalar.dma_start(wo_t[cc][:], woT_v[cc])
        # remaining ctx groups stream on sync/gpsimd alternating by c
        for g in range(2, n_g):
            sc0, nsc = groups[g]
            for c in range(6):
                q = nc.sync if c % 2 == 0 else nc.gpsimd
                q.dma_start(
                    ctx_t[c][g][:], ctxT_v[c][:, sc0 * 128 : (sc0 + nsc) * 128]
                )

        # ---- HAM warm-up ----------------------------------------------------
        # The PE clock sits at 1.2 GHz until ~3.4us of sustained activity.
        # Real work is DMA-gated for the first ~6us after the preamble, so
        # burn that window on dependency-free dummy matmuls: by the time x/wq
        # land, the PE runs at 2.4 GHz and stays there.
        warm_w = const.tile([128, 128], BF16, tag="warm_w")
        nc.vector.memset(warm_w[:], 0.0)
        wps = p_a.tile([128, 512], F32, tag="a", name="warm_ps")
        for i in range(56):
            nc.tensor.matmul(
                wps[:, 0:128], lhsT=warm_w[:], rhs=warm_w[:], start=True, stop=True
            )

        # ---- Q projection (PE warm-up while ctx/wkv stream) -----------------
        # 2 [128,1024] psum tiles; qc regions bank-aligned (one accumulation
        # group per 512-f32 bank).
        qps = [
            p_sc.tile([128, 1024], F32, tag="sc", name=f"qps{i}") for i in range(2)
        ]
        for qc in range(4):
            reg = qps[qc // 2][:, (qc % 2) * 512 : (qc % 2) * 512 + T]
            for ec in range(4):
                nc.tensor.matmul(
                    reg,
                    lhsT=wq_t[ec][:, qc * 128 : (qc + 1) * 128],
                    rhs=x_t[ec][:],
                    start=(ec == 0),
                    stop=(ec == 3),
                )
        for qc in range(4):
            nc.vector.tensor_copy(
                qt_t[qc][:], qps[qc // 2][:, (qc % 2) * 512 : (qc % 2) * 512 + T]
            )

        # ---- group-0 K-part, c-outer so PE starts on first-arrived wkv[c] --
        aps0 = [
            p_sc.tile([128, 1024], F32, tag="sc", name=f"aps0_{i}") for i in range(2)
        ]
        for c in range(6):
            for kc in range(4):
                reg = aps0[kc // 2][:, (kc % 2) * 512 : (kc % 2) * 512 + g0w]
                nc.tensor.matmul(
                    reg,
                    lhsT=wkv_t[c][:, kc * 128 : (kc + 1) * 128],
                    rhs=ctx_t[c][0][:],
                    start=(c == 0),
                    stop=(c == 5),
                )
        for kc in range(4):
            nc.vector.tensor_copy(
                kt_t[kc][:, 0:g0w],
                aps0[kc // 2][:, (kc % 2) * 512 : (kc % 2) * 512 + g0w],
            )

        sc2g = {}
        for gi, (sc0_, nsc_) in enumerate(groups):
            for sc_ in range(sc0_, sc0_ + nsc_):
                sc2g[sc_] = gi

        def vprime(sc):
            """V' for one 128-wide s-chunk: [128 s, 8h*65] with mask folded."""
            g = sc2g[sc]
            off = (sc - groups[g][0]) * 128
            ps = p_a.tile([128, 512], F32, tag="a", name=f"vps{sc}")
            for c in range(6):
                nc.tensor.matmul(
                    ps[:],
                    lhsT=ctx_t[c][g][:, off : off + 128],
                    rhs=wkv_t[c][:, 512:1024],
                    start=(c == 0),
                    stop=(c == 5),
                )
            dst = vp_t[sc][:].rearrange("p (h e) -> p h e", e=65)
            nc.vector.tensor_scalar_mul(
                dst[:, :, 0:64],
                ps[:].rearrange("p (h d) -> p h d", d=64),
                m01_t[:, sc : sc + 1],
            )
            nc.vector.tensor_scalar_mul(
                dst[:, :, 64:65],
                ones8_t[:].rearrange("p (h o) -> p h o", o=1),
                m01_t[:, sc : sc + 1],
            )

        def kpart(g, kc):
            """K-projection c-major slice kc for ctx group g."""
            sc0, nsc = groups[g]
            w = nsc * 128
            ps = p_a.tile([128, 512], F32, tag="a", name=f"kps{g}_{kc}")
            for c in range(6):
                nc.tensor.matmul(
                    ps[:, 0:w],
                    lhsT=wkv_t[c][:, kc * 128 : (kc + 1) * 128],
                    rhs=ctx_t[c][g][:],
                    start=(c == 0),
                    stop=(c == 5),
                )
            nc.vector.tensor_copy(kt_t[kc][:, sc0 * 128 : sc0 * 128 + w], ps[:, 0:w])

        def scores(g, kc):
            """ScoresT + exp for head pair kc, group g. Returns (e0, e1)."""
            sc0, nsc = groups[g]
            w = nsc * 256
            pe0 = p_sc.tile([128, 1024], F32, tag="sc", name=f"pe0_{g}_{kc}")
            pe1 = p_sc.tile([128, 1024], F32, tag="sc", name=f"pe1_{g}_{kc}")
            for j in range(nsc):
                sc = sc0 + j
                nc.tensor.matmul(
                    pe0[:, j * 256 : (j + 1) * 256],
                    lhsT=kt_t[kc][0:64, sc * 128 : (sc + 1) * 128],
                    rhs=qt_t[kc][0:64, :],
                    start=True,
                    stop=True,
                )
                nc.tensor.matmul(
                    pe1[:, j * 256 : (j + 1) * 256],
                    lhsT=kt_t[kc][64:128, sc * 128 : (sc + 1) * 128],
                    rhs=qt_t[kc][64:128, :],
                    start=True,
                    stop=True,
                )
            # exp in <=512-col slices: finer ACT granularity lets the first
            # PV matmuls start before the whole tile is activated.
            e0 = work.tile([128, 1024], BF16, tag="exp", bufs=8, name=f"e0_{g}_{kc}")
            e1 = work.tile([128, 1024], BF16, tag="exp", bufs=8, name=f"e1_{g}_{kc}")
            for lo in range(0, w, 512):
                hi = min(lo + 512, w)
                nc.scalar.activation(
                    e0[:, lo:hi], pe0[:, lo:hi], mybir.ActivationFunctionType.Exp
                )
                nc.scalar.activation(
                    e1[:, lo:hi], pe1[:, lo:hi], mybir.ActivationFunctionType.Exp
                )
            return e0, e1

        def pv(g, kc, e0, e1):
            """PV for head pair kc over group g, accumulate into pvacc."""
            sc0, nsc = groups[g]
            pvq0 = p_pv.tile([65, T], F32, tag="pv", name=f"pvq0_{g}_{kc}")
            pvq1 = p_pv.tile([65, T], F32, tag="pv", name=f"pvq1_{g}_{kc}")
            for j in range(nsc):
                sc = sc0 + j
                nc.tensor.matmul(
                    pvq0[:],
                    lhsT=vp_t[sc][:, (2 * kc) * 65 : (2 * kc) * 65 + 65],
                    rhs=e0[:, j * 256 : (j + 1) * 256],
                    start=(j == 0),
                    stop=(j == nsc - 1),
                )
                nc.tensor.matmul(
                    pvq1[:],
                    lhsT=vp_t[sc][:, (2 * kc + 1) * 65 : (2 * kc + 1) * 65 + 65],
                    rhs=e1[:, j * 256 : (j + 1) * 256],
                    start=(j == 0),
                    stop=(j == nsc - 1),
                )
            if g == 0:
                nc.vector.tensor_copy(pvacc_t[2 * kc][:], pvq0[:])
                nc.vector.tensor_copy(pvacc_t[2 * kc + 1][:], pvq1[:])
            else:
                nc.vector.tensor_add(pvacc_t[2 * kc][:], pvacc_t[2 * kc][:], pvq0[:])
                nc.vector.tensor_add(
                    pvacc_t[2 * kc + 1][:], pvacc_t[2 * kc + 1][:], pvq1[:]
                )

        # out-proj psum: eo regions bank-aligned, accumulated over kc.
        # Allocated lazily at first tail use — allocating earlier would make
        # intermediate scores tiles alias buffers whose consumers (the tail
        # out-proj reads) come later in program order.
        ops = []

        def den_chain(kc):
            """Collect denominators of head pair kc, reciprocal, refold to rech."""
            nc.sync.dma_start(den2_t[kc][0:1, :], pvacc_t[2 * kc][64:65, :])
            nc.gpsimd.dma_start(den2_t[kc][1:2, :], pvacc_t[2 * kc + 1][64:65, :])
            nc.vector.reciprocal_approx_fast(rec2_t[kc][:], den2_t[kc][:])
            nc.sync.dma_start(
                rech_t[0:1, (2 * kc) * T : (2 * kc + 2) * T].rearrange(
                    "p (h t) -> p h t", t=T
                ),
                rec2_t[kc][:],
            )

        def bc_mul(kc):
            """Broadcast 1/den over 64 partitions (K=1 matmul), scale PV -> OT."""
            bc = p_a.tile([128, 512], F32, tag="a", name=f"bc{kc}")
            nc.tensor.matmul(
                bc[0:64, 0:512],
                lhsT=ones64_t[:],
                rhs=rech_t[0:1, (2 * kc) * T : (2 * kc + 2) * T],
                start=True,
                stop=True,
            )
            tmp1 = work.tile([64, T], BF16, tag="otmp", bufs=2, name=f"otmp{kc}")
            nc.vector.tensor_mul(
                tmp1[:], pvacc_t[2 * kc + 1][0:64, :], bc[0:64, T : 2 * T]
            )
            nc.gpsimd.dma_start(ot_t[kc][64:128, :], tmp1[:])
            nc.vector.tensor_mul(
                ot_t[kc][0:64, :], pvacc_t[2 * kc][0:64, :], bc[0:64, 0:T]
            )

        def outproj(kc):
            if not ops:
                ops.append(p_sc.tile([128, 1024], F32, tag="sc", name="ops0"))
                ops.append(p_sc.tile([128, 1024], F32, tag="sc", name="ops1"))
            for eo in range(4):
                reg = ops[eo // 2][:, (eo % 2) * 512 : (eo % 2) * 512 + T]
                nc.tensor.matmul(
                    reg,
                    lhsT=wo_t[kc][:, eo * 128 : (eo + 1) * 128],
                    rhs=ot_t[kc][:],
                    start=(kc == 0),
                    stop=(kc == 3),
                )

        # ---- V'(0) then pipelined groups -----------------------------------
        for sc in range(groups[0][0], groups[0][0] + groups[0][1]):
            vprime(sc)

        exps = {}
        for g in range(1, n_g):
            # scores(g-1) interleaved with K-part(g)
            for kc in range(4):
                exps[(g - 1, kc)] = scores(g - 1, kc)
                kpart(g, kc)
            # PV(g-1) interleaved with V'(g)
            for kc in range(4):
                e0, e1 = exps.pop((g - 1, kc))
                pv(g - 1, kc, e0, e1)
                if kc < groups[g][1]:
                    vprime(groups[g][0] + kc)

        # ---- final group: scores/PV interleaved with the normalization tail.
        # PV(kc)+den_chain(kc) slot between scores issues so the PE never
        # stalls on the den->recip->rech DMA latency; bc/outproj follow once
        # their reciprocals are in flight.
        gl = n_g - 1
        for kc in range(4):
            exps[(gl, kc)] = scores(gl, kc)
            if kc >= 1:
                e0, e1 = exps.pop((gl, kc - 1))
                pv(gl, kc - 1, e0, e1)
                den_chain(kc - 1)
        e0, e1 = exps.pop((gl, 3))
        pv(gl, 3, e0, e1)
        den_chain(3)
        for kc in range(4):
            bc_mul(kc)
        for kc in range(4):
            outproj(kc)

        # ---- bias + output DMA ---------------------------------------------
        for eo in range(4):
            reg = ops[eo // 2][:, (eo % 2) * 512 : (eo % 2) * 512 + T]
            osb = work.tile([128, T], F32, tag="osb", bufs=4, name=f"osb{eo}")
            nc.vector.tensor_scalar_add(osb[:], reg, bo_t[:, eo : eo + 1])
            q = nc.sync if eo % 2 == 0 else nc.gpsimd
            q.dma_start(outT_d[eo], osb[:])

    nc.compile()
    return nc


_NC_CACHE = {}


def _get_nc(s_pad):
    if s_pad not in _NC_CACHE:
        _NC_CACHE[s_pad] = _build_program(s_pad)
    return _NC_CACHE[s_pad]


def _prep_in_maps(x, context, key_padding_mask, Wq, Wkv, Wo, bo):
    keep = [np.flatnonzero(~key_padding_mask[b]) for b in range(B)]
    max_keep = max(len(k) for k in keep)
    s_pad = max(128, -(-max_keep // 128) * 128)

    wqT = (np.ascontiguousarray(Wq.T) * np.float32(D**-0.5)).astype(NPBF16)
    wkvT = np.ascontiguousarray(Wkv.T).astype(NPBF16)
    woT = np.ascontiguousarray(Wo.T).astype(NPBF16)
    bo_r = np.ascontiguousarray(bo.reshape(4, 128).T).astype(np.float32)
    in_maps = []
    for b in range(B):
        nk = len(keep[b])
        ctxc = np.zeros((s_pad, KV), dtype=np.float32)
        ctxc[:nk] = context[b][keep[b]]
        ctxT = np.ascontiguousarray(ctxc.T).astype(NPBF16)
        xT = np.ascontiguousarray(x[b].T).astype(NPBF16)
        m = np.zeros(s_pad, dtype=np.float32)
        m[:nk] = 1.0
        m01 = np.ascontiguousarray(m.reshape(s_pad // 128, 128).T)
        in_maps.append(
            dict(ctxT=ctxT, xT=xT, m01=m01, wqT=wqT, wkvT=wkvT, woT=woT, bo_r=bo_r)
        )
    return in_maps, s_pad


def _run(inputs, trace=False, **kw):
    in_maps, s_pad = _prep_in_maps(**inputs)
    nc = _get_nc(s_pad)
    res = bass_utils.run_bass_kernel_spmd(
        nc, in_maps, core_ids=list(range(NC_CORES)), trace=trace, **kw
    )
    out = np.stack(
        [res.results[b]["outT"].reshape(E, T).T for b in range(B)]
    ).astype(np.float32)
    return out, res


def kernel(**inputs):
    out, _ = _run(inputs, trace=False)
    return out


if __name__ == "__main__":
    rng = np.random.default_rng(0)
    ins = dict(
        x=rng.standard_normal((B, T, E), dtype=np.float32),
        context=rng.standard_normal((B, S, KV), dtype=np.float32),
        key_padding_mask=rng.integers(0, 2, (B, S)).astype(bool),
        Wq=(rng.standard_normal((512, E), dtype=np.float32) * 0.02),
        Wkv=(rng.standard_normal((1024, KV), dtype=np.float32) * 0.02),
        Wo=(rng.standard_normal((E, 512), dtype=np.float32) * 0.02),
        bo=np.zeros(E, dtype=np.float32),
    )
    out = kernel(**ins)
    print("out", out.shape, out.dtype, np.abs(out).mean())
